# revision 1
# baseline (speedup 1.0000x reference)
"""LIF spike (vanilla) Trainium2 kernel.

Reference recurrence over leading time dim T (per element):
    u_t = TAU * u_{t-1} * (1 - o_{t-1}) + x_t
    o_t = (u_t - VTH > 0) ? 1.0 : 0.0

Decomposed into 3 elementwise ops per time step on carried state
c = u * (u <= VTH):
    S1: u = (c mult TAU) add x_t        (scalar_tensor_tensor, skipped at t=0)
    S2: o = relu(u - VTH) > 0           (ACT Relu, bf16 out; host decodes)
    S3: c = (u is_le VTH) mult u        (scalar_tensor_tensor, skipped at t=T-1)

All compares are exact fp32, so the spike train matches the fp32 jax
reference bit-for-bit. On device S2 runs as ACT Relu(u - VTH) with a
bf16 output (any positive fp32 difference survives the downcast as a
positive bf16), halving store traffic; the host maps >0 to 1.0f.

Sharding: pure data parallel over batch dim B=64 -> 8 cores x 8 batches.
Per core: 32MiB in (f32) + 16MiB out (bf16) HBM traffic.
"""

import numpy as np

T = 8
B = 64
C = 128
H = 32
W = 32
NCORES = 8
BS = B // NCORES            # batches per core
N = BS * C * H * W          # 1,048,576 elements per time step per core
P = 128                     # SBUF partitions
F = 4096                    # tile free-dim (tile = [128, 4096] f32 = 2MB)
NCHUNK = N // (P * F)       # spatial chunks per core
TAU = 0.5
VTH = 0.99999

OUT_DTYPE = "bfloat16"      # DRAM spike repr: relu(u-VTH) in bf16; host maps >0 -> 1.0
ACCUM_LOAD = False          # SWDGE accumulating loads (broken at runtime; keep off)


def _build(nt=T, nchunk=NCHUNK, fdim=F, xb=3, ob=3, ub=2, cb=1,
           out_dtype=OUT_DTYPE, accum_load=ACCUM_LOAD):
    import concourse.bacc as bacc
    import concourse.mybir as mybir
    import concourse.tile as tile

    f32 = mybir.dt.float32
    odt = getattr(mybir.dt, out_dtype)
    alu = mybir.AluOpType
    nc = bacc.Bacc("TRN2", target_bir_lowering=False)
    x = nc.dram_tensor("x", [nt, nchunk, P, fdim], f32, kind="ExternalInput")
    o = nc.dram_tensor("o", [nt, nchunk, P, fdim], odt, kind="ExternalOutput")
    s2_act = out_dtype == "bfloat16"
    with tile.TileContext(nc) as tc:
        with (
            tc.tile_pool(name="const", bufs=1) as constp,
            tc.tile_pool(name="xp", bufs=xb) as xp,
            tc.tile_pool(name="opool", bufs=ob) as opl,
            tc.tile_pool(name="up", bufs=ub) as up,
            tc.tile_pool(name="cp", bufs=cb) as cp,
        ):
            nvth = constp.tile([P, 1], f32)
            nc.vector.memset(nvth[:], -VTH)
            for i in range(nchunk):
                ct = None
                for t in range(nt):
                    if t == 0:
                        u = xp.tile([P, fdim], f32)
                        nc.sync.dma_start(u[:], x[t, i])
                    elif accum_load:
                        # u := tau*c, then DMA adds x_t in-flight (CCE add)
                        u = up.tile([P, fdim], f32)
                        nc.vector.tensor_scalar_mul(u[:], ct[:], TAU)
                        nc.gpsimd.dma_start(u[:], x[t, i], accum_op=alu.add)
                    else:
                        xt = xp.tile([P, fdim], f32)
                        nc.sync.dma_start(xt[:], x[t, i])
                        u = up.tile([P, fdim], f32)
                        nc.vector.scalar_tensor_tensor(
                            u[:], ct[:], TAU, xt[:], alu.mult, alu.add
                        )
                    ot = opl.tile([P, fdim], odt)
                    if s2_act:
                        # spike iff relu(u - VTH) > 0; exact in fp32, and any
                        # positive fp32 survives the bf16 downcast as positive
                        nc.scalar.activation(
                            ot[:], u[:], mybir.ActivationFunctionType.Relu,
                            bias=nvth[:], scale=1.0,
                        )
                    else:
                        nc.vector.tensor_scalar(ot[:], u[:], VTH, None, alu.is_gt)
                    nc.sync.dma_start(o[t, i], ot[:])
                    if t < nt - 1:
                        ct = cp.tile([P, fdim], f32)
                        nc.vector.scalar_tensor_tensor(
                            ct[:], u[:], VTH, u[:], alu.is_le, alu.mult
                        )
    nc.finalize()
    return nc


def kernel(x):
    x = np.ascontiguousarray(np.asarray(x, dtype=np.float32))
    assert x.shape == (T, B, C, H, W), x.shape
    from concourse.bass_utils import run_bass_kernel_spmd

    nc = _build()
    in_maps = []
    for i in range(NCORES):
        s = np.ascontiguousarray(x[:, i * BS : (i + 1) * BS])
        in_maps.append({"x": s.reshape(T, NCHUNK, P, F)})
    res = run_bass_kernel_spmd(nc, in_maps, core_ids=list(range(NCORES)))
    out = np.empty((T, B, C, H, W), np.float32)
    for i, r in enumerate(res.results):
        out[:, i * BS : (i + 1) * BS] = _decode(r["o"]).reshape(T, BS, C, H, W)
    return out


def _decode(o):
    """Device spike repr -> f32 spike train (bf16 relu(u-VTH): spike iff >0)."""
    o = np.asarray(o)
    if o.dtype == np.float32:
        return o
    return (o > 0).astype(np.float32)



# revision 11
# speedup vs baseline: 1.2041x; 1.2041x over previous
"""LIF spike (vanilla) Trainium2 kernel — time-packed spike output.

Reference recurrence over leading time dim T (per element):
    u_t = TAU * u_{t-1} * (1 - o_{t-1}) + x_t
    o_t = (u_t - VTH > 0) ? 1.0 : 0.0

Per time step on carried state c = u * (u <= VTH):
    S1 (DVE):  u = (c mult TAU) add x_t          scalar_tensor_tensor, skipped at t=0
    S2 (ACT):  s = Sign(u - VTH)  in {-1,0,+1}   bf16 out
    S3 (PE):   psum += 2^t * s                   scaled-identity matmul, accumulate
    S4 (DVE):  c = (u is_le VTH) mult u          skipped at t=T-1

After the 8 steps of a chunk, ACT decodes psum -> packed byte value
    b = 0.5*psum + 127.5  in {0..255}  (bit t of b == o_t), stored as
bf16 (integers <=255 are exact in bf16).  Host expands bits -> f32.

All fp32 compare/arith exactly matches the jax fp32 reference (mult by
TAU=0.5 and by 0/1 masks is exact; Sign sees the same fp32 u - VTH).
The only inexact corner is u == VTH exactly (Sign=0 corrupts one byte);
probability ~1e-8/element.

Sharding: pure data parallel over batch dim B=64 -> 8 cores x 8 batches.
Per core HBM traffic: 32 MiB in (f32) + 2 MiB out (packed bf16).
"""

import numpy as np

T = 8
B = 64
C = 128
H = 32
W = 32
NCORES = 8
BS = B // NCORES            # batches per core
N = BS * C * H * W          # 1,048,576 elements per time step per core
P = 128                     # SBUF partitions
F = 4096                    # tile free-dim
NCHUNK = N // (P * F)       # spatial chunks per core
TAU = 0.5
VTH = 0.99999
MMF = 512                   # matmul moving free dim (= one PSUM bank of f32)

PACK_DTYPE = "bfloat16"     # packed byte value dtype in DRAM
GP_COLS = 0                 # columns of each STT offloaded to gpsimd (0 = none)


def _build(nt=T, nchunk=NCHUNK, fdim=F, xb=3, ob=2, ub=2, cb=2,
           pack_dtype=PACK_DTYPE, gp_cols=GP_COLS):
    import concourse.bacc as bacc
    import concourse.mybir as mybir
    import concourse.tile as tile

    f32 = mybir.dt.float32
    bf16 = mybir.dt.bfloat16
    pdt = getattr(mybir.dt, pack_dtype)
    alu = mybir.AluOpType
    act = mybir.ActivationFunctionType
    nbank = fdim // MMF
    nc = bacc.Bacc("TRN2", target_bir_lowering=False)
    x = nc.dram_tensor("x", [nt, nchunk, P, fdim], f32, kind="ExternalInput")
    w = nc.dram_tensor("w", [P, nt, P], bf16, kind="ExternalInput")
    o = nc.dram_tensor("o", [nchunk, P, fdim], pdt, kind="ExternalOutput")

    def stt(eng_cols, out, in0, scalar, in1, op0, op1):
        # split one elementwise op between DVE and (optionally) gpsimd
        if gp_cols:
            dv = fdim - gp_cols
            nc.vector.scalar_tensor_tensor(
                out[:, :dv], in0[:, :dv], scalar, in1[:, :dv], op0, op1)
            nc.gpsimd.scalar_tensor_tensor(
                out[:, dv:], in0[:, dv:], scalar, in1[:, dv:], op0, op1)
        else:
            nc.vector.scalar_tensor_tensor(out[:], in0[:], scalar, in1[:], op0, op1)

    with tile.TileContext(nc) as tc:
        with (
            tc.tile_pool(name="wp", bufs=1) as wp,
            tc.tile_pool(name="xp", bufs=xb) as xp,
            tc.tile_pool(name="op", bufs=ob) as opl,
            tc.tile_pool(name="up", bufs=ub) as up,
            tc.tile_pool(name="cp", bufs=cb) as cp,
            tc.tile_pool(name="pk", bufs=2) as pk,
            tc.tile_pool(name="ps", bufs=1, space="PSUM") as ps,
        ):
            wt = wp.tile([P, nt, P], bf16)
            nc.sync.dma_start(wt[:], w[:])
            nvth = wp.tile([P, 1], f32)
            nc.vector.memset(nvth[:], -VTH)
            dbias = wp.tile([P, 1], f32)
            nc.vector.memset(dbias[:], 127.5)
            for i in range(nchunk):
                ct = None
                pst = [ps.tile([P, MMF], f32, name=f"ps{j}", tag=f"ps{j}")
                       for j in range(nbank)]
                for t in range(nt):
                    if t == 0:
                        u = xp.tile([P, fdim], f32)
                        nc.sync.dma_start(u[:], x[t, i])
                    else:
                        xt = xp.tile([P, fdim], f32)
                        nc.sync.dma_start(xt[:], x[t, i])
                        u = up.tile([P, fdim], f32)
                        stt(None, u, ct, TAU, xt, alu.mult, alu.add)
                    st = opl.tile([P, fdim], bf16)
                    nc.scalar.activation(st[:], u[:], act.Sign, bias=nvth[:], scale=1.0)
                    for j in range(nbank):
                        nc.tensor.matmul(
                            pst[j][:], wt[:, t], st[:, j * MMF:(j + 1) * MMF],
                            start=(t == 0), stop=(t == nt - 1),
                        )
                    if t < nt - 1:
                        ct = cp.tile([P, fdim], f32)
                        stt(None, ct, u, VTH, u, alu.is_le, alu.mult)
                pkt = pk.tile([P, fdim], pdt)
                for j in range(nbank):
                    nc.scalar.activation(
                        pkt[:, j * MMF:(j + 1) * MMF], pst[j][:],
                        act.Identity, bias=dbias[:], scale=0.5,
                    )
                nc.sync.dma_start(o[i], pkt[:])
    nc.finalize()
    return nc


def _weights():
    # lhsT layout [K=P, t, M=P]: w[k, t, m] = 2^t * (k == m)
    w = np.zeros((P, T, P), np.float32)
    for t in range(T):
        np.fill_diagonal(w[:, t, :], float(2 ** t))
    import ml_dtypes

    return w.astype(ml_dtypes.bfloat16)


def _decode(o):
    """Packed byte values -> f32 spike train [T, ...]."""
    o = np.asarray(o)
    if o.dtype != np.float32:  # bf16 arrives as uint16 bits or ml_dtypes
        if o.dtype == np.uint16:
            o = (o.astype(np.uint32) << 16).view(np.float32)
        else:
            o = o.astype(np.float32)
    b = np.rint(o).astype(np.uint8)
    out = np.empty((T,) + b.shape, np.float32)
    for t in range(T):
        out[t] = ((b >> t) & 1).astype(np.float32)
    return out


def make_in_maps(x):
    wb = _weights()
    in_maps = []
    for i in range(NCORES):
        s = np.ascontiguousarray(x[:, i * BS: (i + 1) * BS])
        in_maps.append({"x": s.reshape(T, NCHUNK, P, F), "w": wb})
    return in_maps


def kernel(x):
    x = np.ascontiguousarray(np.asarray(x, dtype=np.float32))
    assert x.shape == (T, B, C, H, W), x.shape
    from concourse.bass_utils import run_bass_kernel_spmd

    nc = _build()
    res = run_bass_kernel_spmd(nc, make_in_maps(x), core_ids=list(range(NCORES)))
    out = np.empty((T, B, C, H, W), np.float32)
    for i, r in enumerate(res.results):
        out[:, i * BS: (i + 1) * BS] = _decode(r["o"]).reshape(T, BS, C, H, W)
    return out
